# revision 60
# baseline (speedup 1.0000x reference)
"""GCN (4-layer, improved self-loops) on 8 Trainium2 NeuronCores.

Sharding: 1D node partition (6250 nodes/core); edges partitioned by
destination-node owner; per layer the raw features t = h@Wg are AllGathered
into a full bf16 DRAM table on every core, then each core gathers per-edge
source rows with dma_gather and scatter-adds them into per-destination-block
PSUM tiles via one-hot matmuls on the TensorEngine.

The full GCN normalization (w_e * dinv[src] * dinv[dst], and the self-loop
coefficient 2*dinv^2) is folded into the per-edge one-hot weights on the
host, so no on-chip pre/post scaling is needed:
    h_next = elu(sum_e w'_e * t[src_e] + b)
Self-loop contributions use the node-major t tiles already resident in SBUF
as scatter lhsT directly (no DMA gather, no table read).

The gather table is fp8-e4m3 in a 256B-stride padded row layout (rows in
natural node order): each gather descriptor reads only the 128B payload,
halving the per-descriptor DMA cost vs bf16 rows (the cost model charges a
2x latency multiplier below 512B, so descriptor size, not bandwidth, is
what counts).  Each dst block's table slab is DMAed to DRAM in pairs right
after its epilogue (512B descriptors), so the AllGather can start as soon
as the last block closes.  Gather indices are int16; the remote table is
addressed by row-parity lo/hi streams via pair indices + elem_step.

Scatter weights: a fixed ~55% subset of the one-hot sw tiles (identical
across layers) is host-prebuilt in fp8 and kept resident in SBUF for the
whole run; the rest are built per layer on DVE as bf16.  Matmuls run fp8
stationary x bf16-or-fp8 moving (1 cycle/row either way); self-loop tiles
stay bf16 (node-major t in SBUF).  Epilogues trail the matmul stream on a
catch-up schedule so the in-order DVE queue never blocks on PSUM waits at
layer boundaries.
"""

import inspect
import numpy as np
from contextlib import ExitStack

try:
    import concourse.bass as bass
except ImportError:  # pragma: no cover
    import sys

    sys.path.insert(0, "/opt/trn_rl_repo")
    import concourse.bass as bass

import concourse.bacc as bacc
import concourse.mybir as mybir
import concourse.tile as tile
from concourse.bass_utils import run_bass_kernel_spmd

FP = mybir.dt.float32
BF = mybir.dt.bfloat16
F8 = mybir.dt.float8e4
I16 = mybir.dt.int16

# dma_gather with the elem-size granularity relaxed from 256B to 128B.  The
# 256B restriction is only on the descriptor's read length; the source-row
# STRIDE stays 256B-encoded (stride_bytes_256), which the padded fp8 table
# layouts below satisfy.  Verified bit-exact on hardware (probe_fp8.py).
_src = inspect.getsource(bass.BassGpSimd.dma_gather)
assert "% 256 == 0" in _src
_src = "def _dma_gather_128" + _src[len("    def dma_gather"):].replace(
    "% 256 == 0", "% 128 == 0"
).replace("\n    ", "\n")
_ns = dict(vars(bass))
exec(_src, _ns)
_dma_gather_128 = _ns["_dma_gather_128"]

N = 50000
E = 800000
IN_D = 64
H = 128
OUT_D = 16
P = 8
NC_N = N // P            # 6250 nodes per core
BW = 128                 # destination-block width (scatter matmul moving dim)
NBLK = -(-NC_N // BW)    # 49 destination blocks per core
NPAD = NBLK * BW         # 6272
SPLIT = 32768            # lo/hi split of permuted tfull rows (int16 indices)
C_TILES = 32             # 128-edge tiles per dma_gather call
EP_START_AHEAD = 6       # blocks of sw/matmuls emitted before epilogue 0
EP_LAG_MIN = 2           # steady-state epilogue lag after catch-up
EP_L0_MULT = 3           # longer start-ahead for layer 0 (MLP-tail boundary)
SW_RESIDENT = 560        # leading meta tiles kept resident in SBUF as fp8,
                         # host-precomputed, reused by all 4 layers
SW_POOL_EVERY = 10 ** 9  # disabled: anything chained through the Pool
                         # queue mid-layer picks up desc-gen-sized latency

# dense-matmul column chunks over the padded node dim
CHUNKS = [(k * 512, 512) for k in range(12)] + [(6144, 128)]

ALU = mybir.AluOpType
ACT_F = mybir.ActivationFunctionType


def _resident_set(TT):
    """Meta-tile ids kept resident (host-prebuilt fp8).  A solid lead run
    bridges the layer-boundary bubble (no DVE work needed while the table
    emit + first gathers land); the rest is spread evenly so DVE sw builds
    and gather DMA overlap through the whole layer instead of phase-locking
    into a DMA-only region followed by a DVE-only region."""
    R = min(SW_RESIDENT, TT)
    lead = min(120, R)
    ids = set(range(lead))
    rest = R - lead
    if rest > 0 and TT > lead:
        ids.update(int(x) for x in np.linspace(lead, TT - 1, rest))
    ids = sorted(ids)
    return ids, {g: i for i, g in enumerate(ids)}


def _rowperm_local(m):
    """Table row of local node m.  Blocks are emitted to DRAM in PAIRS with
    one linear [128, 2x512B] copy (512B descriptors, full DMA rate): the
    copy lands partition p's two 256B halves (block 2q node p, block 2q+1
    node p) on consecutive rows, so the row of node m in pair region q is
    q*256 + 2*(m%128) + block-parity.  The trailing unpaired block (NBLK
    odd) keeps identity rows."""
    m = np.asarray(m)
    pair_end = (NBLK // 2) * 256
    q, r = m // 256, m % 256
    perm = q * 256 + 2 * (r % 128) + r // 128
    return np.where(m < pair_end, perm, m)


def _layout(nmax):
    """Dense per-stream slot layout shared by host prep and program build.

    Each (block, stream) group occupies slots [O[s][b], O[s][b]+nmax[s,b])
    of its stream — no tile alignment, so gather descriptors cover only the
    max-over-cores real edge count.  A 128-slot tile straddling a block
    boundary is consumed by both blocks (with w=0 masks from meta).

    Returns (O, ST, NT, uses, meta_of, TT): slot offsets, stream slot
    totals, stream tile counts, per-block list of (stream, tile) uses, the
    meta tile index of each block's self tile, and the total meta tiles.
    """
    O = [np.concatenate([[0], np.cumsum(nmax[s])]) for s in range(3)]
    ST = [int(O[s][-1]) for s in range(3)]
    NT = [-(-ST[s] // 128) for s in range(3)]
    uses = []
    meta_of = []
    om = 0
    for b in range(NBLK):
        meta_of.append(om)
        om += 1
        ub = []
        for s in range(3):
            n = int(nmax[s][b])
            if n == 0:
                continue
            t0 = int(O[s][b]) // 128
            t1 = int(O[s][b] + n - 1) // 128
            ub.extend((s, t) for t in range(t0, t1 + 1))
        om += len(ub)
        uses.append(ub)
    return O, ST, NT, uses, meta_of, om


def _prep_edges(edge_index, edge_weight):
    """Host preprocessing: partition edges by dst owner, fold the full GCN
    normalization into per-edge weights, split local/remote-lo/remote-hi by
    source table row, group by BW-dst block, pad each (core, block, stream)
    group to a common (max-over-cores) tile count.

    Returns (tcnt, per_core); per_core[c] has lcidx/loidx/hiidx/meta arrays.
    """
    import ml_dtypes

    src = np.asarray(edge_index[0], dtype=np.int64)
    dst = np.asarray(edge_index[1], dtype=np.int64)
    w = np.asarray(edge_weight, dtype=np.float32)

    core = dst // NC_N
    drel = dst % NC_N

    deg_full = np.zeros(N, dtype=np.float64)
    np.add.at(deg_full, dst, w.astype(np.float64))
    dinv_full = (1.0 / np.sqrt(deg_full + 2.0)).astype(np.float32)

    wn = w * dinv_full[src] * dinv_full[dst]   # folded edge norm

    # permuted global table row for source node s
    src_core = src // NC_N
    src_m = src % NC_N
    row_local = _rowperm_local(src_m)
    row_global = src_core * NPAD + row_local

    groups = [[[None] * 3 for _ in range(NBLK)] for _ in range(P)]
    for c in range(P):
        mask = core == c
        s_core, d_all, w_all = src_core[mask], drel[mask], wn[mask]
        rg, rl = row_global[mask], row_local[mask]
        blk = d_all // BW
        rel = (d_all % BW).astype(np.float32)
        is_local = s_core == c
        even = rg % 2 == 0
        for b in range(NBLK):
            mb = blk == b
            # remote edges split by table-row parity: parity is uniform
            # across cores, so the max-over-cores stream sizes stay balanced
            # (an absolute row split would skew per core); idx = row//2 fits
            # int16 since the table is viewed as [rows/2, 2H] row pairs.
            for s, ms in (
                (0, mb & is_local),
                (1, mb & ~is_local & even),
                (2, mb & ~is_local & ~even),
            ):
                idx = rl[ms] if s == 0 else rg[ms] // 2
                # sort by source row for DRAM locality
                o = np.argsort(idx, kind="stable")
                groups[c][b][s] = (
                    idx[o].astype(np.int16), rel[ms][o], w_all[ms][o],
                )

    nmax = np.zeros((3, NBLK), dtype=np.int64)
    for b in range(NBLK):
        for s in range(3):
            for c in range(P):
                nmax[s, b] = max(nmax[s, b], len(groups[c][b][s][0]))

    O, ST, NT, uses, meta_of, TT = _layout(nmax)

    per_core = []
    for c in range(P):
        dinv_c = np.zeros(NPAD, dtype=np.float32)
        dinv_c[:NC_N] = dinv_full[c * NC_N : (c + 1) * NC_N]
        selfw = 2.0 * dinv_c * dinv_c

        idx_bufs = [np.zeros(NT[s] * 128, dtype=np.int16) for s in range(3)]
        # meta[e] = (dst_rel, w) per consumption-order tile USE: per block,
        # the self tile first, then each stream tile overlapping the block's
        # dense slot range; out-of-range rows keep w=0.
        meta_rel = np.zeros(TT * 128, dtype=np.float32)
        meta_w = np.zeros(TT * 128, dtype=np.float32)
        om = 0
        iota128 = np.arange(128, dtype=np.float32)
        for b in range(NBLK):
            meta_rel[om * 128 : om * 128 + 128] = iota128
            meta_w[om * 128 : om * 128 + 128] = selfw[b * 128 : (b + 1) * 128]
            om += 1
            for s, T in uses[b]:
                idx, rel, ww = groups[c][b][s]
                n = len(idx)
                o_sb = int(O[s][b])
                if T == o_sb // 128:
                    idx_bufs[s][o_sb : o_sb + n] = idx
                lo_s = max(o_sb, 128 * T)
                hi_s = min(o_sb + n, 128 * (T + 1))
                if hi_s > lo_s:
                    mo = om * 128 + (lo_s - 128 * T)
                    meta_rel[mo : mo + hi_s - lo_s] = rel[lo_s - o_sb : hi_s - o_sb]
                    meta_w[mo : mo + hi_s - lo_s] = ww[lo_s - o_sb : hi_s - o_sb]
                om += 1

        # wrapped int16 index layout: idx i lives at [i % 16, i // 16],
        # replicated 8x along partitions (one stripe per Q7 core)
        wraps = [
            np.ascontiguousarray(np.tile(ib.reshape(-1, 16).T, (8, 1)))
            if len(ib)
            else np.zeros((128, 0), dtype=np.int16)
            for ib in idx_bufs
        ]
        # meta in partition-major tile layout holds ONLY the streamed
        # (non-resident) tiles, in streamed-rank order
        res_ids0, _ = _resident_set(TT)
        stream_ids = [g for g in range(TT) if g not in set(res_ids0)]
        NS = len(stream_ids)
        mr = meta_rel.reshape(TT, 128)[stream_ids]
        mw = meta_w.reshape(TT, 128)[stream_ids]
        meta = np.empty((128, 2 * NS), dtype=np.float32)
        meta[:, 0::2] = mr.T
        meta[:, 1::2] = mw.T

        # resident sw tiles, prebuilt as fp8 one-hot [128 slot, BW dst]
        # scatter weights and kept in SBUF for all layers
        res_ids, _res_rank = _resident_set(TT)
        rel_all = meta_rel.reshape(TT, 128).astype(np.int64)
        w_all2 = meta_w.reshape(TT, 128)
        swc = np.zeros((128, len(res_ids) * BW), dtype=np.float32)
        rows = np.arange(128)
        for i, g in enumerate(res_ids):
            swc[rows, i * BW + rel_all[g]] = w_all2[g]

        per_core.append(
            {
                "lcidx": wraps[0],
                "loidx": wraps[1],
                "hiidx": wraps[2],
                "meta": meta,
                "swc": swc.astype(ml_dtypes.float8_e4m3),
            }
        )

    return nmax, per_core


def _build_program(tcnt, single_core=False):
    # single_core=True swaps the AllGather for a local DMA copy and builds a
    # 1-device module, so the cost-model TimelineSim (single-core only) can
    # profile the kernel; numerics of remote nodes are wrong in that mode.
    nmax = tcnt
    O, ST, NT, uses, meta_of, TT = _layout(nmax)
    TS = NT
    nc = bacc.Bacc(
        "TRN2",
        target_bir_lowering=False,
        debug=False,
        enable_asserts=False,
        num_devices=1 if single_core else P,
    )

    # ---- I/O -------------------------------------------------------------
    xT_d = nc.dram_tensor("xT", [IN_D, NC_N], BF, kind="ExternalInput")
    lcidx_d = nc.dram_tensor("lcidx", [128, max(TS[0], 1) * 8], I16, kind="ExternalInput")
    loidx_d = nc.dram_tensor("loidx", [128, max(TS[1], 1) * 8], I16, kind="ExternalInput")
    hiidx_d = nc.dram_tensor("hiidx", [128, max(TS[2], 1) * 8], I16, kind="ExternalInput")
    res_ids, res_rank = _resident_set(TT)
    R = len(res_ids)
    stream_rank = {}
    for g in range(TT):
        if g not in res_rank:
            stream_rank[g] = len(stream_rank)
    NS = len(stream_rank)
    meta_d = nc.dram_tensor("meta", [128, 2 * NS], FP, kind="ExternalInput")
    swc_d = nc.dram_tensor("swc", [128, R * BW], F8, kind="ExternalInput")
    w_d = {
        name: nc.dram_tensor(name, shape, BF, kind="ExternalInput")
        for name, shape in [
            ("W1", [IN_D, H]),
            ("W2", [H, H]),
            ("W3", [H, H]),
            ("Wg1", [H, H]),
            ("Wg2", [H, H]),
            ("Wg3", [H, H]),
            ("Wg4", [H, H]),
            ("Wh", [H, OUT_D]),
        ]
    }
    # bias columns: 0..2 = b1..b3, 3..6 = bg1..bg4, 7..13 = negated, 14 = bh
    bias_d = nc.dram_tensor("bias", [128, 24], FP, kind="ExternalInput")
    iota_d = nc.dram_tensor("iota128", [128, BW], BF, kind="ExternalInput")
    out_d = nc.dram_tensor("out", [OUT_D, NC_N], FP, kind="ExternalOutput")

    rg = [list(range(P))]

    with tile.TileContext(nc) as tc, ExitStack() as ctx:
        const = ctx.enter_context(tc.tile_pool(name="const", bufs=1))
        big = ctx.enter_context(tc.tile_pool(name="big", bufs=1))
        swp = ctx.enter_context(tc.tile_pool(name="swp", bufs=84))
        epp = ctx.enter_context(tc.tile_pool(name="epp", bufs=10))
        rp_p = ctx.enter_context(tc.tile_pool(name="rp", bufs=6))
        e2_p = ctx.enter_context(tc.tile_pool(name="e2", bufs=6))
        vlc_p = ctx.enter_context(tc.tile_pool(name="vlc", bufs=2))
        vlo_p = ctx.enter_context(tc.tile_pool(name="vlo", bufs=3))
        vhi_p = ctx.enter_context(tc.tile_pool(name="vhi", bufs=3))
        ps_dense = ctx.enter_context(tc.tile_pool(name="psd", bufs=2, space="PSUM"))
        ps_blk = ctx.enter_context(
            tc.tile_pool(name="psb", bufs=4, space="PSUM")
        )
        ps_tr = ctx.enter_context(tc.tile_pool(name="pst", bufs=2, space="PSUM"))
        oc_p = ctx.enter_context(tc.tile_pool(name="oc", bufs=3))
        t8p = ctx.enter_context(tc.tile_pool(name="t8p", bufs=6))
        dram = ctx.enter_context(tc.tile_pool(name="dram", bufs=2, space="DRAM"))

        # ---- constants ----------------------------------------------------
        def load_const(shape, src_ap, name, dtype=FP):
            t = const.tile(shape, dtype, tag=name)
            nc.sync.dma_start(t[:], src_ap)
            return t

        w_sb = {k: load_const(list(v.shape), v[:], k, BF) for k, v in w_d.items()}
        bias = load_const([128, 24], bias_d[:], "bias")
        iota = load_const([128, BW], iota_d[:], "iota", BF)
        h_sb = big.tile([128, NPAD], BF, tag="h")
        tt_sb = big.tile([128, NPAD], BF, tag="tt")
        xc = big.tile([IN_D, NPAD], BF, tag="xc")

        nc.vector.memset(xc[:, NC_N:], 0.0)
        nc.sync.dma_start(xc[:, :NC_N], xT_d[:])

        # bulk constants (meta/swc/idx: ~8MB) load AFTER xc so the first W1
        # matmul isn't queued behind them; none are read before layer 0
        meta_sb = load_const([128, 2 * NS], meta_d[:], "meta")
        swc = load_const([128, R * BW], swc_d[:], "swc", F8)
        idx_sb = [
            load_const([128, max(TS[s], 1) * 8], d[:], f"idx{s}", I16)
            for s, d in enumerate((lcidx_d, loidx_d, hiidx_d))
        ]

        sw_count = [0]

        def sw_tile(g):
            """[128 edge, BW dst] one-hot(dst_rel)*w scatter tile for
            consumption-order tile g.  Mostly built on the vector engine;
            every SW_POOL_EVERY-th tile goes to gpsimd to balance DVE/Pool
            occupancy (gpsimd is ~3x slower per tile but otherwise idle
            between gather descriptor generations)."""
            i = res_rank.get(g)
            if i is not None:
                return swc[:, i * BW : (i + 1) * BW]
            g = stream_rank[g]
            sw = swp.tile([128, BW], BF, tag="sw")
            eng = (
                nc.gpsimd
                if sw_count[0] % SW_POOL_EVERY == SW_POOL_EVERY - 1
                else nc.vector
            )
            sw_count[0] += 1
            eng.tensor_scalar(
                sw[:],
                iota[:],
                meta_sb[:, 2 * g : 2 * g + 1],
                meta_sb[:, 2 * g + 1 : 2 * g + 2],
                ALU.is_equal,
                ALU.mult,
            )
            return sw[:]

        agin_next = [None]
        pend_t8 = [None]

        def emit_t(b0, b1, wg, on_dve=False):
            # t (node-major) = h_blk^T @ Wg per block; lhsT = h slice puts
            # nodes on the output partition axis, so no transposes needed.
            # Each block is written twice: bf16 to SBUF (self tiles) + fp8
            # padded rows straight to the next layer's DRAM table slab
            # [b*128, (b+1)*128) — one 32KB descriptor per block, landing as
            # soon as the block's h is final (no end-of-layer staged copy).
            # During the MLP phase ACT is the pacer and DVE idles, so the
            # copies go to DVE there (on_dve); in-layer it's the reverse.
            for b in range(b0, b1):
                trp = ps_tr.tile([128, 128], FP, tag="tr")
                nc.tensor.matmul(trp[:], h_sb[:, b * BW : (b + 1) * BW], wg[:])
                nc.scalar.activation(
                    tt_sb[:, b * BW : (b + 1) * BW], trp[:], ACT_F.Copy
                )
                # pairs of blocks share one [128, 2x512B] staging tile and
                # flush with a single linear copy on odd b; the row layout
                # (_rowperm_local) is defined so this lands every node on its
                # table row with full-rate 512B descriptors
                if b % 2 == 0 and b < NBLK - 1:
                    t8r_new = t8p.tile([128, 4 * BW], F8, tag="t8r")
                    pend_t8[0] = t8r_new
                    t8r, half = t8r_new, 0
                elif b % 2 == 1:
                    t8r, half = pend_t8[0], 2 * BW
                else:  # trailing unpaired block
                    t8r_new = t8p.tile([128, 4 * BW], F8, tag="t8r")
                    t8r, half = t8r_new, 0
                if on_dve:
                    # table-gating fp8 copy on DVE: during the MLP tail the
                    # ACT queue is the pacer and would delay the first
                    # gathers; the bf16 self-tile copy above is not urgent
                    nc.vector.tensor_scalar(
                        t8r[:, half : half + BW], trp[:], 0.0, None, ALU.add
                    )
                else:
                    nc.scalar.activation(
                        t8r[:, half : half + BW], trp[:], ACT_F.Copy
                    )
                if b % 2 == 1:
                    nc.sync.dma_start(
                        agin_next[0][(b - 1) * 128 : (b + 1) * 128, :].bitcast(
                            BF
                        ),
                        t8r[:].bitcast(BF),
                    )
                elif b == NBLK - 1:
                    nc.sync.dma_start(
                        agin_next[0][b * 128 : (b + 1) * 128, :],
                        t8r[:, : 2 * BW],
                    )

        agin0 = dram.tile([NPAD, 2 * H], F8, tag="agin")
        agin_next[0] = agin0

        # ---- embedding MLP -------------------------------------------------

        # ELU via  elu(z) = min(exp(z), 1) + max(z-1, -1)  (exact for all z):
        # exp on ACT, the shifted relu and combine on DVE.  Stage-split loops
        # keep each in-order engine queue free of cross-chunk dependency
        # chains.  The W3 combine loop interleaves layer 1's t-matmuls so the
        # first GCN table emit isn't serialized behind the whole MLP.
        for wname, bcol in [("W1", 0), ("W2", 1), ("W3", 2)]:
            # W1/W2 store h+1 = min(exp(z),1) + relu(z): both pieces come off
            # ACT and the offset is pre-folded into the next bias.  W3 output
            # feeds the GCN, so it uses the exact form with the shifted relu
            # on DVE.
            offset_form = wname != "W3"
            rps, e2s = [], []
            for off, cw in CHUNKS:
                ps = ps_dense.tile([128, 512], FP, tag="dense")
                if wname == "W1":
                    nc.tensor.matmul(
                        ps[:, :cw], w_sb["W1"][:IN_D, :], xc[:IN_D, off : off + cw]
                    )
                else:
                    nc.tensor.matmul(
                        ps[:, :cw], w_sb[wname][:], h_sb[:, off : off + cw]
                    )
                rp = rp_p.tile([128, 512], BF, tag="rp")
                if offset_form:
                    nc.scalar.activation(
                        rp[:, :cw], ps[:, :cw], ACT_F.Relu,
                        bias=bias[:, bcol : bcol + 1],
                    )
                else:
                    nc.vector.tensor_scalar(
                        rp[:, :cw], ps[:, :cw], bias[:, bcol + 16 : bcol + 17],
                        -1.0, ALU.add, ALU.max,
                    )
                e2 = e2_p.tile([128, 512], BF, tag="e2")
                nc.scalar.activation(
                    e2[:, :cw], ps[:, :cw], ACT_F.Exp,
                    bias=bias[:, bcol : bcol + 1],
                )
                rps.append(rp)
                e2s.append(e2)
            for ci, (off, cw) in enumerate(CHUNKS):
                nc.vector.scalar_tensor_tensor(
                    h_sb[:, off : off + cw], e2s[ci][:, :cw], 1.0,
                    rps[ci][:, :cw], ALU.min, ALU.add,
                )
                if wname == "W3":
                    emit_t(off // BW, min(NBLK, (off + cw) // BW),
                           w_sb["Wg1"], on_dve=True)

        # ---- GCN layers ---------------------------------------------------
        cstarts = []
        for s in range(3):
            sizes, rem = [], NT[s]
            # small first chunk: the first v tiles of every stream gate block
            # 0's matmuls right after the table lands, so minimize their
            # desc-gen + transfer latency
            first = min(8, rem)
            if first:
                sizes.append(first)
                rem -= first
            while rem > C_TILES + C_TILES // 2:
                sizes.append(C_TILES)
                rem -= C_TILES
            if rem > C_TILES // 2:
                sizes.extend([(rem + 1) // 2, rem // 2])
            elif rem:
                sizes.append(rem)
            cstarts.append(np.concatenate([[0], np.cumsum(sizes)]).astype(int))
        n_chunk = [len(cstarts[s]) - 1 for s in range(3)]
        # emit gather chunks interleaved by first consuming block
        chunk_order = sorted(
            (max(0, int(np.searchsorted(O[s], int(cstarts[s][ci]) * 128,
                                        "right")) - 1), s, ci)
            for s in range(3) for ci in range(n_chunk[s])
        )

        for layer in range(4):
            wg = w_sb[f"Wg{layer + 1}"]
            bcol = 3 + layer
            # layer 0's t is emitted inside the MLP W3 combine loop; later
            # layers' t blocks are emitted inside the PREVIOUS layer's
            # epilogues, right after each h block is finalized
            next_wg = w_sb[f"Wg{layer + 2}"] if layer < 3 else None

            # agin was filled block-by-block by the previous layer's
            # epilogues (rows in natural node order, [NPAD, 256B] padded);
            # tfull is viewed as [row-pairs, 512B] so the even/odd gather
            # streams address all P*NPAD rows with int16 pair indices +
            # elem_step, reading 128B payloads.
            agin = agin_next[0]
            tfull = dram.tile(
                [P * NPAD // 2, 4 * H], F8, tag="tfull", addr_space="Shared"
            )

            if single_core:
                nc.sync.dma_start(tfull[: NPAD // 2, :], agin[:, :])
            else:
                nc.gpsimd.collective_compute(
                    "AllGather",
                    ALU.bypass,
                    replica_groups=rg,
                    ins=[agin[:]],
                    outs=[tfull[:]],
                )

            tables = (agin[:, :H], tfull[:, :H], tfull[:, 2 * H : 3 * H])
            steps = (2 * H, 4 * H, 4 * H)
            vpools = (vlc_p, vlo_p, vhi_p)
            # chunked gathers over the dense slot streams.  Every call runs
            # at full num_idxs: the idx buffers are zero-padded past ST[s],
            # so trailing slots fetch table row 0 (valid bytes, w=0 masks in
            # meta) — no memset, which would head-of-line block the DVE
            # queue on v-pool recycling at layer boundaries.
            vchunks = [[None] * n_chunk[s] for s in range(3)]
            for _, s, ci in chunk_order:
                t0 = int(cstarts[s][ci])
                nt = int(cstarts[s][ci + 1]) - t0
                v = vpools[s].tile([128, C_TILES, 128], F8, tag=f"v{s}")
                _dma_gather_128(
                    nc.gpsimd, v[:, :nt, :], tables[s],
                    idx_sb[s][:, t0 * 8 : (t0 + nt) * 8],
                    nt * 128, nt * 128, H, elem_step=steps[s],
                    single_packet=False,
                )
                vchunks[s][ci] = v

            # per-block scatter-accumulate; the self tile (SBUF node-major t)
            # leads each block's accumulation group.  Epilogues trail the
            # matmul stream by EP_LOOKAHEAD blocks so the in-order DVE queue
            # never head-of-line blocks upcoming sw builds on a PSUM wait.
            aggs = {}

            def do_block(b):
                ntile = 1 + len(uses[b])
                agg = ps_blk.tile([128, BW], FP, tag="agg")
                sw = sw_tile(meta_of[b])
                nc.tensor.matmul(
                    agg[:], tt_sb[:, b * BW : (b + 1) * BW], sw,
                    start=True, stop=(ntile == 1),
                )
                for t, (s, T) in enumerate(uses[b], start=1):
                    sw = sw_tile(meta_of[b] + t)
                    ci = int(np.searchsorted(cstarts[s], T, "right")) - 1
                    v = vchunks[s][ci][:, T - int(cstarts[s][ci]), :]
                    nc.tensor.matmul(
                        agg[:], v, sw,
                        start=False, stop=(t == ntile - 1),
                    )
                aggs[b] = agg

            def do_epilogue(b):
                agg = aggs.pop(b)
                rp = epp.tile([128, BW], FP, tag="rpb")
                nc.vector.tensor_scalar(
                    rp[:], agg[:], bias[:, bcol + 16 : bcol + 17],
                    -1.0, ALU.add, ALU.max,
                )
                eb = epp.tile([128, BW], BF, tag="eb")
                nc.scalar.activation(
                    eb[:], agg[:], ACT_F.Exp, bias=bias[:, bcol : bcol + 1]
                )
                nc.vector.scalar_tensor_tensor(
                    h_sb[:, b * BW : (b + 1) * BW],
                    eb[:], 1.0, rp[:], ALU.min, ALU.add,
                )
                if next_wg is not None:
                    emit_t(b, b + 1, next_wg)

            if next_wg is not None:
                agin_nl = dram.tile([NPAD, 2 * H], F8, tag="agin")
                agin_next[0] = agin_nl

            # catch-up epilogue schedule: a large lag at layer start keeps
            # DVE building sw through the table-emit/gather-latency bubble;
            # then epilogues catch up 2-per-block to a small steady lag so
            # the PSUM agg ring and swp ring never back-pressure mid-layer.
            emitted = 0
            ep_a = EP_START_AHEAD * (EP_L0_MULT if layer == 0 else 1)
            for b in range(NBLK):
                do_block(b)
                if b >= ep_a:
                    tgt = b - EP_LAG_MIN + 1
                    for _ in range(max(0, min(3, tgt - emitted))):
                        do_epilogue(emitted)
                        emitted += 1
            while emitted < NBLK:
                do_epilogue(emitted)
                emitted += 1

        # ---- head ----------------------------------------------------------
        for off, cw in CHUNKS:
            cw = min(cw, NC_N - off)
            ps = ps_dense.tile([128, 512], FP, tag="dense")
            nc.tensor.matmul(
                ps[:OUT_D, :cw], w_sb["Wh"][:], h_sb[:, off : off + cw]
            )
            ot = oc_p.tile([OUT_D, 512], FP, tag="ot")
            nc.scalar.activation(
                ot[:, :cw], ps[:OUT_D, :cw], ACT_F.Identity,
                bias=bias[:OUT_D, 14:15],
            )
            nc.sync.dma_start(out_d[:, off : off + cw], ot[:, :cw])

    nc.compile()
    return nc


def _make_in_maps(inputs, per_core):
    import ml_dtypes

    x = np.asarray(inputs["x"], dtype=np.float32)
    # the W1/W2 MLP layers store h+1 (ELU plus one); the constant offset is
    # folded into the consuming layer's bias via column sums of the bf16
    # weights actually used on device
    w2bf = np.asarray(inputs["W2"], np.float32).astype(ml_dtypes.bfloat16)
    w3bf = np.asarray(inputs["W3"], np.float32).astype(ml_dtypes.bfloat16)
    bias = np.zeros((128, 24), dtype=np.float32)
    for j, nm in enumerate(["b1", "b2", "b3", "bg1", "bg2", "bg3", "bg4"]):
        b = np.asarray(inputs[nm], dtype=np.float32)
        bias[:, j] = b
        bias[:, j + 16] = b - 1.0
    bias[:, 1] -= w2bf.astype(np.float32).sum(axis=0)
    bias[:, 2] -= w3bf.astype(np.float32).sum(axis=0)
    bias[:, 17] = bias[:, 1] - 1.0
    bias[:, 18] = bias[:, 2] - 1.0
    bias[:OUT_D, 14] = np.asarray(inputs["bh"], dtype=np.float32)

    shared = {
        "bias": bias,
        "iota128": np.tile(
            np.arange(BW, dtype=np.float32), (128, 1)
        ).astype(ml_dtypes.bfloat16),
    }
    for nm in ["W1", "W2", "W3", "Wg1", "Wg2", "Wg3", "Wg4", "Wh"]:
        shared[nm] = np.ascontiguousarray(
            np.asarray(inputs[nm], np.float32)
        ).astype(ml_dtypes.bfloat16)

    in_maps = []
    for c in range(P):
        m = dict(shared)
        m["xT"] = np.ascontiguousarray(
            x[c * NC_N : (c + 1) * NC_N].T
        ).astype(ml_dtypes.bfloat16)
        m.update(per_core[c])
        in_maps.append(m)
    return in_maps


def run(inputs, trace=False):
    """Run the distributed kernel; returns (out [N, OUT_D] fp32, results)."""
    tcnt, per_core = _prep_edges(inputs["edge_index"], inputs["edge_weight"])
    nc = _build_program(tcnt)
    in_maps = _make_in_maps(inputs, per_core)
    res = run_bass_kernel_spmd(nc, in_maps, list(range(P)), trace=trace)
    out = np.concatenate(
        [res.results[c]["out"].T for c in range(P)], axis=0
    ).astype(np.float32)
    return out, res


def kernel(**inputs):
    out, _ = run(inputs, trace=False)
    return out



# revision 61
# speedup vs baseline: 1.0105x; 1.0105x over previous
"""GCN (4-layer, improved self-loops) on 8 Trainium2 NeuronCores.

Sharding: 1D node partition (6250 nodes/core); edges partitioned by
destination-node owner; per layer the raw features t = h@Wg are AllGathered
into a full bf16 DRAM table on every core, then each core gathers per-edge
source rows with dma_gather and scatter-adds them into per-destination-block
PSUM tiles via one-hot matmuls on the TensorEngine.

The full GCN normalization (w_e * dinv[src] * dinv[dst], and the self-loop
coefficient 2*dinv^2) is folded into the per-edge one-hot weights on the
host, so no on-chip pre/post scaling is needed:
    h_next = elu(sum_e w'_e * t[src_e] + b)
Self-loop contributions use the node-major t tiles already resident in SBUF
as scatter lhsT directly (no DMA gather, no table read).

The gather table is fp8-e4m3 in a 256B-stride padded row layout (rows in
natural node order): each gather descriptor reads only the 128B payload,
halving the per-descriptor DMA cost vs bf16 rows (the cost model charges a
2x latency multiplier below 512B, so descriptor size, not bandwidth, is
what counts).  Each dst block's table slab is DMAed to DRAM in pairs right
after its epilogue (512B descriptors), so the AllGather can start as soon
as the last block closes.  Gather indices are int16; the remote table is
addressed by row-parity lo/hi streams via pair indices + elem_step.

Scatter weights: a fixed ~55% subset of the one-hot sw tiles (identical
across layers) is host-prebuilt in fp8 and kept resident in SBUF for the
whole run; the rest are built per layer on DVE as bf16.  Matmuls run fp8
stationary x bf16-or-fp8 moving (1 cycle/row either way); self-loop tiles
stay bf16 (node-major t in SBUF).  Epilogues trail the matmul stream on a
catch-up schedule so the in-order DVE queue never blocks on PSUM waits at
layer boundaries.
"""

import inspect
import numpy as np
from contextlib import ExitStack

try:
    import concourse.bass as bass
except ImportError:  # pragma: no cover
    import sys

    sys.path.insert(0, "/opt/trn_rl_repo")
    import concourse.bass as bass

import concourse.bacc as bacc
import concourse.mybir as mybir
import concourse.tile as tile
from concourse.bass_utils import run_bass_kernel_spmd

FP = mybir.dt.float32
BF = mybir.dt.bfloat16
F8 = mybir.dt.float8e4
I16 = mybir.dt.int16

# dma_gather with the elem-size granularity relaxed from 256B to 128B.  The
# 256B restriction is only on the descriptor's read length; the source-row
# STRIDE stays 256B-encoded (stride_bytes_256), which the padded fp8 table
# layouts below satisfy.  Verified bit-exact on hardware (probe_fp8.py).
_src = inspect.getsource(bass.BassGpSimd.dma_gather)
assert "% 256 == 0" in _src
_src = "def _dma_gather_128" + _src[len("    def dma_gather"):].replace(
    "% 256 == 0", "% 128 == 0"
).replace("\n    ", "\n")
_ns = dict(vars(bass))
exec(_src, _ns)
_dma_gather_128 = _ns["_dma_gather_128"]

N = 50000
E = 800000
IN_D = 64
H = 128
OUT_D = 16
P = 8
NC_N = N // P            # 6250 nodes per core
BW = 128                 # destination-block width (scatter matmul moving dim)
NBLK = -(-NC_N // BW)    # 49 destination blocks per core
NPAD = NBLK * BW         # 6272
SPLIT = 32768            # lo/hi split of permuted tfull rows (int16 indices)
C_TILES = 24             # 128-edge tiles per dma_gather call
EP_START_AHEAD = 6       # blocks of sw/matmuls emitted before epilogue 0
EP_LAG_MIN = 2           # steady-state epilogue lag after catch-up
EP_L0_MULT = 3           # longer start-ahead for layer 0 (MLP-tail boundary)
SW_RESIDENT = 560        # leading meta tiles kept resident in SBUF as fp8,
                         # host-precomputed, reused by all 4 layers
SW_POOL_EVERY = 10 ** 9  # disabled: anything chained through the Pool
                         # queue mid-layer picks up desc-gen-sized latency

# dense-matmul column chunks over the padded node dim
CHUNKS = [(k * 512, 512) for k in range(12)] + [(6144, 128)]

ALU = mybir.AluOpType
ACT_F = mybir.ActivationFunctionType


def _resident_set(TT):
    """Meta-tile ids kept resident (host-prebuilt fp8).  A solid lead run
    bridges the layer-boundary bubble (no DVE work needed while the table
    emit + first gathers land); the rest is spread evenly so DVE sw builds
    and gather DMA overlap through the whole layer instead of phase-locking
    into a DMA-only region followed by a DVE-only region."""
    R = min(SW_RESIDENT, TT)
    lead = min(120, R)
    ids = set(range(lead))
    rest = R - lead
    if rest > 0 and TT > lead:
        ids.update(int(x) for x in np.linspace(lead, TT - 1, rest))
    ids = sorted(ids)
    return ids, {g: i for i, g in enumerate(ids)}


def _rowperm_local(m):
    """Table row of local node m.  Blocks are emitted to DRAM in PAIRS with
    one linear [128, 2x512B] copy (512B descriptors, full DMA rate): the
    copy lands partition p's two 256B halves (block 2q node p, block 2q+1
    node p) on consecutive rows, so the row of node m in pair region q is
    q*256 + 2*(m%128) + block-parity.  The trailing unpaired block (NBLK
    odd) keeps identity rows."""
    m = np.asarray(m)
    pair_end = (NBLK // 2) * 256
    q, r = m // 256, m % 256
    perm = q * 256 + 2 * (r % 128) + r // 128
    return np.where(m < pair_end, perm, m)


def _layout(nmax):
    """Dense per-stream slot layout shared by host prep and program build.

    Each (block, stream) group occupies slots [O[s][b], O[s][b]+nmax[s,b])
    of its stream — no tile alignment, so gather descriptors cover only the
    max-over-cores real edge count.  A 128-slot tile straddling a block
    boundary is consumed by both blocks (with w=0 masks from meta).

    Returns (O, ST, NT, uses, meta_of, TT): slot offsets, stream slot
    totals, stream tile counts, per-block list of (stream, tile) uses, the
    meta tile index of each block's self tile, and the total meta tiles.
    """
    O = [np.concatenate([[0], np.cumsum(nmax[s])]) for s in range(3)]
    ST = [int(O[s][-1]) for s in range(3)]
    NT = [-(-ST[s] // 128) for s in range(3)]
    uses = []
    meta_of = []
    om = 0
    for b in range(NBLK):
        meta_of.append(om)
        om += 1
        ub = []
        for s in range(3):
            n = int(nmax[s][b])
            if n == 0:
                continue
            t0 = int(O[s][b]) // 128
            t1 = int(O[s][b] + n - 1) // 128
            ub.extend((s, t) for t in range(t0, t1 + 1))
        om += len(ub)
        uses.append(ub)
    return O, ST, NT, uses, meta_of, om


def _prep_edges(edge_index, edge_weight):
    """Host preprocessing: partition edges by dst owner, fold the full GCN
    normalization into per-edge weights, split local/remote-lo/remote-hi by
    source table row, group by BW-dst block, pad each (core, block, stream)
    group to a common (max-over-cores) tile count.

    Returns (tcnt, per_core); per_core[c] has lcidx/loidx/hiidx/meta arrays.
    """
    import ml_dtypes

    src = np.asarray(edge_index[0], dtype=np.int64)
    dst = np.asarray(edge_index[1], dtype=np.int64)
    w = np.asarray(edge_weight, dtype=np.float32)

    core = dst // NC_N
    drel = dst % NC_N

    deg_full = np.zeros(N, dtype=np.float64)
    np.add.at(deg_full, dst, w.astype(np.float64))
    dinv_full = (1.0 / np.sqrt(deg_full + 2.0)).astype(np.float32)

    wn = w * dinv_full[src] * dinv_full[dst]   # folded edge norm

    # permuted global table row for source node s
    src_core = src // NC_N
    src_m = src % NC_N
    row_local = _rowperm_local(src_m)
    row_global = src_core * NPAD + row_local

    groups = [[[None] * 3 for _ in range(NBLK)] for _ in range(P)]
    for c in range(P):
        mask = core == c
        s_core, d_all, w_all = src_core[mask], drel[mask], wn[mask]
        rg, rl = row_global[mask], row_local[mask]
        blk = d_all // BW
        rel = (d_all % BW).astype(np.float32)
        is_local = s_core == c
        even = rg % 2 == 0
        for b in range(NBLK):
            mb = blk == b
            # remote edges split by table-row parity: parity is uniform
            # across cores, so the max-over-cores stream sizes stay balanced
            # (an absolute row split would skew per core); idx = row//2 fits
            # int16 since the table is viewed as [rows/2, 2H] row pairs.
            for s, ms in (
                (0, mb & is_local),
                (1, mb & ~is_local & even),
                (2, mb & ~is_local & ~even),
            ):
                idx = rl[ms] if s == 0 else rg[ms] // 2
                # sort by source row for DRAM locality
                o = np.argsort(idx, kind="stable")
                groups[c][b][s] = (
                    idx[o].astype(np.int16), rel[ms][o], w_all[ms][o],
                )

    nmax = np.zeros((3, NBLK), dtype=np.int64)
    for b in range(NBLK):
        for s in range(3):
            for c in range(P):
                nmax[s, b] = max(nmax[s, b], len(groups[c][b][s][0]))

    O, ST, NT, uses, meta_of, TT = _layout(nmax)

    per_core = []
    for c in range(P):
        dinv_c = np.zeros(NPAD, dtype=np.float32)
        dinv_c[:NC_N] = dinv_full[c * NC_N : (c + 1) * NC_N]
        selfw = 2.0 * dinv_c * dinv_c

        idx_bufs = [np.zeros(NT[s] * 128, dtype=np.int16) for s in range(3)]
        # meta[e] = (dst_rel, w) per consumption-order tile USE: per block,
        # the self tile first, then each stream tile overlapping the block's
        # dense slot range; out-of-range rows keep w=0.
        meta_rel = np.zeros(TT * 128, dtype=np.float32)
        meta_w = np.zeros(TT * 128, dtype=np.float32)
        om = 0
        iota128 = np.arange(128, dtype=np.float32)
        for b in range(NBLK):
            meta_rel[om * 128 : om * 128 + 128] = iota128
            meta_w[om * 128 : om * 128 + 128] = selfw[b * 128 : (b + 1) * 128]
            om += 1
            for s, T in uses[b]:
                idx, rel, ww = groups[c][b][s]
                n = len(idx)
                o_sb = int(O[s][b])
                if T == o_sb // 128:
                    idx_bufs[s][o_sb : o_sb + n] = idx
                lo_s = max(o_sb, 128 * T)
                hi_s = min(o_sb + n, 128 * (T + 1))
                if hi_s > lo_s:
                    mo = om * 128 + (lo_s - 128 * T)
                    meta_rel[mo : mo + hi_s - lo_s] = rel[lo_s - o_sb : hi_s - o_sb]
                    meta_w[mo : mo + hi_s - lo_s] = ww[lo_s - o_sb : hi_s - o_sb]
                om += 1

        # wrapped int16 index layout: idx i lives at [i % 16, i // 16],
        # replicated 8x along partitions (one stripe per Q7 core)
        wraps = [
            np.ascontiguousarray(np.tile(ib.reshape(-1, 16).T, (8, 1)))
            if len(ib)
            else np.zeros((128, 0), dtype=np.int16)
            for ib in idx_bufs
        ]
        # meta in partition-major tile layout holds ONLY the streamed
        # (non-resident) tiles, in streamed-rank order
        res_ids0, _ = _resident_set(TT)
        stream_ids = [g for g in range(TT) if g not in set(res_ids0)]
        NS = len(stream_ids)
        mr = meta_rel.reshape(TT, 128)[stream_ids]
        mw = meta_w.reshape(TT, 128)[stream_ids]
        meta = np.empty((128, 2 * NS), dtype=np.float32)
        meta[:, 0::2] = mr.T
        meta[:, 1::2] = mw.T

        # resident sw tiles, prebuilt as fp8 one-hot [128 slot, BW dst]
        # scatter weights and kept in SBUF for all layers
        res_ids, _res_rank = _resident_set(TT)
        rel_all = meta_rel.reshape(TT, 128).astype(np.int64)
        w_all2 = meta_w.reshape(TT, 128)
        swc = np.zeros((128, len(res_ids) * BW), dtype=np.float32)
        rows = np.arange(128)
        for i, g in enumerate(res_ids):
            swc[rows, i * BW + rel_all[g]] = w_all2[g]

        per_core.append(
            {
                "lcidx": wraps[0],
                "loidx": wraps[1],
                "hiidx": wraps[2],
                "meta": meta,
                "swc": swc.astype(ml_dtypes.float8_e4m3),
            }
        )

    return nmax, per_core


def _build_program(tcnt, single_core=False):
    # single_core=True swaps the AllGather for a local DMA copy and builds a
    # 1-device module, so the cost-model TimelineSim (single-core only) can
    # profile the kernel; numerics of remote nodes are wrong in that mode.
    nmax = tcnt
    O, ST, NT, uses, meta_of, TT = _layout(nmax)
    TS = NT
    nc = bacc.Bacc(
        "TRN2",
        target_bir_lowering=False,
        debug=False,
        enable_asserts=False,
        num_devices=1 if single_core else P,
    )

    # ---- I/O -------------------------------------------------------------
    xT_d = nc.dram_tensor("xT", [IN_D, NC_N], BF, kind="ExternalInput")
    lcidx_d = nc.dram_tensor("lcidx", [128, max(TS[0], 1) * 8], I16, kind="ExternalInput")
    loidx_d = nc.dram_tensor("loidx", [128, max(TS[1], 1) * 8], I16, kind="ExternalInput")
    hiidx_d = nc.dram_tensor("hiidx", [128, max(TS[2], 1) * 8], I16, kind="ExternalInput")
    res_ids, res_rank = _resident_set(TT)
    R = len(res_ids)
    stream_rank = {}
    for g in range(TT):
        if g not in res_rank:
            stream_rank[g] = len(stream_rank)
    NS = len(stream_rank)
    meta_d = nc.dram_tensor("meta", [128, 2 * NS], FP, kind="ExternalInput")
    swc_d = nc.dram_tensor("swc", [128, R * BW], F8, kind="ExternalInput")
    w_d = {
        name: nc.dram_tensor(name, shape, BF, kind="ExternalInput")
        for name, shape in [
            ("W1", [IN_D, H]),
            ("W2", [H, H]),
            ("W3", [H, H]),
            ("Wg1", [H, H]),
            ("Wg2", [H, H]),
            ("Wg3", [H, H]),
            ("Wg4", [H, H]),
            ("Wh", [H, OUT_D]),
        ]
    }
    # bias columns: 0..2 = b1..b3, 3..6 = bg1..bg4, 7..13 = negated, 14 = bh
    bias_d = nc.dram_tensor("bias", [128, 24], FP, kind="ExternalInput")
    iota_d = nc.dram_tensor("iota128", [128, BW], BF, kind="ExternalInput")
    out_d = nc.dram_tensor("out", [OUT_D, NC_N], FP, kind="ExternalOutput")

    rg = [list(range(P))]

    with tile.TileContext(nc) as tc, ExitStack() as ctx:
        const = ctx.enter_context(tc.tile_pool(name="const", bufs=1))
        big = ctx.enter_context(tc.tile_pool(name="big", bufs=1))
        swp = ctx.enter_context(tc.tile_pool(name="swp", bufs=80))
        epp = ctx.enter_context(tc.tile_pool(name="epp", bufs=10))
        rp_p = ctx.enter_context(tc.tile_pool(name="rp", bufs=6))
        e2_p = ctx.enter_context(tc.tile_pool(name="e2", bufs=6))
        vlc_p = ctx.enter_context(tc.tile_pool(name="vlc", bufs=3))
        vlo_p = ctx.enter_context(tc.tile_pool(name="vlo", bufs=4))
        vhi_p = ctx.enter_context(tc.tile_pool(name="vhi", bufs=4))
        ps_dense = ctx.enter_context(tc.tile_pool(name="psd", bufs=2, space="PSUM"))
        ps_blk = ctx.enter_context(
            tc.tile_pool(name="psb", bufs=4, space="PSUM")
        )
        ps_tr = ctx.enter_context(tc.tile_pool(name="pst", bufs=2, space="PSUM"))
        oc_p = ctx.enter_context(tc.tile_pool(name="oc", bufs=3))
        t8p = ctx.enter_context(tc.tile_pool(name="t8p", bufs=6))
        dram = ctx.enter_context(tc.tile_pool(name="dram", bufs=2, space="DRAM"))

        # ---- constants ----------------------------------------------------
        def load_const(shape, src_ap, name, dtype=FP):
            t = const.tile(shape, dtype, tag=name)
            nc.sync.dma_start(t[:], src_ap)
            return t

        w_sb = {k: load_const(list(v.shape), v[:], k, BF) for k, v in w_d.items()}
        bias = load_const([128, 24], bias_d[:], "bias")
        iota = load_const([128, BW], iota_d[:], "iota", BF)
        h_sb = big.tile([128, NPAD], BF, tag="h")
        tt_sb = big.tile([128, NPAD], BF, tag="tt")
        xc = big.tile([IN_D, NPAD], BF, tag="xc")

        nc.vector.memset(xc[:, NC_N:], 0.0)
        nc.sync.dma_start(xc[:, :NC_N], xT_d[:])

        # bulk constants (meta/swc/idx: ~8MB) load AFTER xc so the first W1
        # matmul isn't queued behind them; none are read before layer 0
        meta_sb = load_const([128, 2 * NS], meta_d[:], "meta")
        swc = load_const([128, R * BW], swc_d[:], "swc", F8)
        idx_sb = [
            load_const([128, max(TS[s], 1) * 8], d[:], f"idx{s}", I16)
            for s, d in enumerate((lcidx_d, loidx_d, hiidx_d))
        ]

        sw_count = [0]

        def sw_tile(g):
            """[128 edge, BW dst] one-hot(dst_rel)*w scatter tile for
            consumption-order tile g.  Mostly built on the vector engine;
            every SW_POOL_EVERY-th tile goes to gpsimd to balance DVE/Pool
            occupancy (gpsimd is ~3x slower per tile but otherwise idle
            between gather descriptor generations)."""
            i = res_rank.get(g)
            if i is not None:
                return swc[:, i * BW : (i + 1) * BW]
            g = stream_rank[g]
            sw = swp.tile([128, BW], BF, tag="sw")
            eng = (
                nc.gpsimd
                if sw_count[0] % SW_POOL_EVERY == SW_POOL_EVERY - 1
                else nc.vector
            )
            sw_count[0] += 1
            eng.tensor_scalar(
                sw[:],
                iota[:],
                meta_sb[:, 2 * g : 2 * g + 1],
                meta_sb[:, 2 * g + 1 : 2 * g + 2],
                ALU.is_equal,
                ALU.mult,
            )
            return sw[:]

        agin_next = [None]
        pend_t8 = [None]

        def emit_t(b0, b1, wg, on_dve=False):
            # t (node-major) = h_blk^T @ Wg per block; lhsT = h slice puts
            # nodes on the output partition axis, so no transposes needed.
            # Each block is written twice: bf16 to SBUF (self tiles) + fp8
            # padded rows straight to the next layer's DRAM table slab
            # [b*128, (b+1)*128) — one 32KB descriptor per block, landing as
            # soon as the block's h is final (no end-of-layer staged copy).
            # During the MLP phase ACT is the pacer and DVE idles, so the
            # copies go to DVE there (on_dve); in-layer it's the reverse.
            for b in range(b0, b1):
                trp = ps_tr.tile([128, 128], FP, tag="tr")
                nc.tensor.matmul(trp[:], h_sb[:, b * BW : (b + 1) * BW], wg[:])
                nc.scalar.activation(
                    tt_sb[:, b * BW : (b + 1) * BW], trp[:], ACT_F.Copy
                )
                # pairs of blocks share one [128, 2x512B] staging tile and
                # flush with a single linear copy on odd b; the row layout
                # (_rowperm_local) is defined so this lands every node on its
                # table row with full-rate 512B descriptors
                if b % 2 == 0 and b < NBLK - 1:
                    t8r_new = t8p.tile([128, 4 * BW], F8, tag="t8r")
                    pend_t8[0] = t8r_new
                    t8r, half = t8r_new, 0
                elif b % 2 == 1:
                    t8r, half = pend_t8[0], 2 * BW
                else:  # trailing unpaired block
                    t8r_new = t8p.tile([128, 4 * BW], F8, tag="t8r")
                    t8r, half = t8r_new, 0
                if on_dve:
                    # table-gating fp8 copy on DVE: during the MLP tail the
                    # ACT queue is the pacer and would delay the first
                    # gathers; the bf16 self-tile copy above is not urgent
                    nc.vector.tensor_scalar(
                        t8r[:, half : half + BW], trp[:], 0.0, None, ALU.add
                    )
                else:
                    nc.scalar.activation(
                        t8r[:, half : half + BW], trp[:], ACT_F.Copy
                    )
                if b % 2 == 1:
                    nc.sync.dma_start(
                        agin_next[0][(b - 1) * 128 : (b + 1) * 128, :].bitcast(
                            BF
                        ),
                        t8r[:].bitcast(BF),
                    )
                elif b == NBLK - 1:
                    nc.sync.dma_start(
                        agin_next[0][b * 128 : (b + 1) * 128, :],
                        t8r[:, : 2 * BW],
                    )

        agin0 = dram.tile([NPAD, 2 * H], F8, tag="agin")
        agin_next[0] = agin0

        # ---- embedding MLP -------------------------------------------------

        # ELU via  elu(z) = min(exp(z), 1) + max(z-1, -1)  (exact for all z):
        # exp on ACT, the shifted relu and combine on DVE.  Stage-split loops
        # keep each in-order engine queue free of cross-chunk dependency
        # chains.  The W3 combine loop interleaves layer 1's t-matmuls so the
        # first GCN table emit isn't serialized behind the whole MLP.
        for wname, bcol in [("W1", 0), ("W2", 1), ("W3", 2)]:
            # W1/W2 store h+1 = min(exp(z),1) + relu(z): both pieces come off
            # ACT and the offset is pre-folded into the next bias.  W3 output
            # feeds the GCN, so it uses the exact form with the shifted relu
            # on DVE.
            offset_form = wname != "W3"
            rps, e2s = [], []
            for off, cw in CHUNKS:
                ps = ps_dense.tile([128, 512], FP, tag="dense")
                if wname == "W1":
                    nc.tensor.matmul(
                        ps[:, :cw], w_sb["W1"][:IN_D, :], xc[:IN_D, off : off + cw]
                    )
                else:
                    nc.tensor.matmul(
                        ps[:, :cw], w_sb[wname][:], h_sb[:, off : off + cw]
                    )
                rp = rp_p.tile([128, 512], BF, tag="rp")
                if offset_form:
                    nc.scalar.activation(
                        rp[:, :cw], ps[:, :cw], ACT_F.Relu,
                        bias=bias[:, bcol : bcol + 1],
                    )
                else:
                    nc.vector.tensor_scalar(
                        rp[:, :cw], ps[:, :cw], bias[:, bcol + 16 : bcol + 17],
                        -1.0, ALU.add, ALU.max,
                    )
                e2 = e2_p.tile([128, 512], BF, tag="e2")
                nc.scalar.activation(
                    e2[:, :cw], ps[:, :cw], ACT_F.Exp,
                    bias=bias[:, bcol : bcol + 1],
                )
                rps.append(rp)
                e2s.append(e2)
            for ci, (off, cw) in enumerate(CHUNKS):
                nc.vector.scalar_tensor_tensor(
                    h_sb[:, off : off + cw], e2s[ci][:, :cw], 1.0,
                    rps[ci][:, :cw], ALU.min, ALU.add,
                )
                if wname == "W3":
                    emit_t(off // BW, min(NBLK, (off + cw) // BW),
                           w_sb["Wg1"], on_dve=True)

        # ---- GCN layers ---------------------------------------------------
        cstarts = []
        for s in range(3):
            sizes, rem = [], NT[s]
            # small first chunk: the first v tiles of every stream gate block
            # 0's matmuls right after the table lands, so minimize their
            # desc-gen + transfer latency
            first = min(8, rem)
            if first:
                sizes.append(first)
                rem -= first
            while rem > C_TILES + C_TILES // 2:
                sizes.append(C_TILES)
                rem -= C_TILES
            if rem > C_TILES // 2:
                sizes.extend([(rem + 1) // 2, rem // 2])
            elif rem:
                sizes.append(rem)
            cstarts.append(np.concatenate([[0], np.cumsum(sizes)]).astype(int))
        n_chunk = [len(cstarts[s]) - 1 for s in range(3)]
        # emit gather chunks interleaved by first consuming block
        chunk_order = sorted(
            (max(0, int(np.searchsorted(O[s], int(cstarts[s][ci]) * 128,
                                        "right")) - 1), s, ci)
            for s in range(3) for ci in range(n_chunk[s])
        )

        for layer in range(4):
            wg = w_sb[f"Wg{layer + 1}"]
            bcol = 3 + layer
            # layer 0's t is emitted inside the MLP W3 combine loop; later
            # layers' t blocks are emitted inside the PREVIOUS layer's
            # epilogues, right after each h block is finalized
            next_wg = w_sb[f"Wg{layer + 2}"] if layer < 3 else None

            # agin was filled block-by-block by the previous layer's
            # epilogues (rows in natural node order, [NPAD, 256B] padded);
            # tfull is viewed as [row-pairs, 512B] so the even/odd gather
            # streams address all P*NPAD rows with int16 pair indices +
            # elem_step, reading 128B payloads.
            agin = agin_next[0]
            tfull = dram.tile(
                [P * NPAD // 2, 4 * H], F8, tag="tfull", addr_space="Shared"
            )

            if single_core:
                nc.sync.dma_start(tfull[: NPAD // 2, :], agin[:, :])
            else:
                nc.gpsimd.collective_compute(
                    "AllGather",
                    ALU.bypass,
                    replica_groups=rg,
                    ins=[agin[:]],
                    outs=[tfull[:]],
                )

            tables = (agin[:, :H], tfull[:, :H], tfull[:, 2 * H : 3 * H])
            steps = (2 * H, 4 * H, 4 * H)
            vpools = (vlc_p, vlo_p, vhi_p)
            # chunked gathers over the dense slot streams.  Every call runs
            # at full num_idxs: the idx buffers are zero-padded past ST[s],
            # so trailing slots fetch table row 0 (valid bytes, w=0 masks in
            # meta) — no memset, which would head-of-line block the DVE
            # queue on v-pool recycling at layer boundaries.
            vchunks = [[None] * n_chunk[s] for s in range(3)]
            for _, s, ci in chunk_order:
                t0 = int(cstarts[s][ci])
                nt = int(cstarts[s][ci + 1]) - t0
                v = vpools[s].tile([128, C_TILES, 128], F8, tag=f"v{s}")
                _dma_gather_128(
                    nc.gpsimd, v[:, :nt, :], tables[s],
                    idx_sb[s][:, t0 * 8 : (t0 + nt) * 8],
                    nt * 128, nt * 128, H, elem_step=steps[s],
                    single_packet=False,
                )
                vchunks[s][ci] = v

            # per-block scatter-accumulate; the self tile (SBUF node-major t)
            # leads each block's accumulation group.  Epilogues trail the
            # matmul stream by EP_LOOKAHEAD blocks so the in-order DVE queue
            # never head-of-line blocks upcoming sw builds on a PSUM wait.
            aggs = {}

            def do_block(b):
                ntile = 1 + len(uses[b])
                agg = ps_blk.tile([128, BW], FP, tag="agg")
                sw = sw_tile(meta_of[b])
                nc.tensor.matmul(
                    agg[:], tt_sb[:, b * BW : (b + 1) * BW], sw,
                    start=True, stop=(ntile == 1),
                )
                for t, (s, T) in enumerate(uses[b], start=1):
                    sw = sw_tile(meta_of[b] + t)
                    ci = int(np.searchsorted(cstarts[s], T, "right")) - 1
                    v = vchunks[s][ci][:, T - int(cstarts[s][ci]), :]
                    nc.tensor.matmul(
                        agg[:], v, sw,
                        start=False, stop=(t == ntile - 1),
                    )
                aggs[b] = agg

            def do_epilogue(b):
                agg = aggs.pop(b)
                rp = epp.tile([128, BW], FP, tag="rpb")
                nc.vector.tensor_scalar(
                    rp[:], agg[:], bias[:, bcol + 16 : bcol + 17],
                    -1.0, ALU.add, ALU.max,
                )
                eb = epp.tile([128, BW], BF, tag="eb")
                nc.scalar.activation(
                    eb[:], agg[:], ACT_F.Exp, bias=bias[:, bcol : bcol + 1]
                )
                nc.vector.scalar_tensor_tensor(
                    h_sb[:, b * BW : (b + 1) * BW],
                    eb[:], 1.0, rp[:], ALU.min, ALU.add,
                )
                if next_wg is not None:
                    emit_t(b, b + 1, next_wg)

            if next_wg is not None:
                agin_nl = dram.tile([NPAD, 2 * H], F8, tag="agin")
                agin_next[0] = agin_nl

            # catch-up epilogue schedule: a large lag at layer start keeps
            # DVE building sw through the table-emit/gather-latency bubble;
            # then epilogues catch up 2-per-block to a small steady lag so
            # the PSUM agg ring and swp ring never back-pressure mid-layer.
            emitted = 0
            ep_a = EP_START_AHEAD * (EP_L0_MULT if layer == 0 else 1)
            for b in range(NBLK):
                do_block(b)
                if b >= ep_a:
                    tgt = b - EP_LAG_MIN + 1
                    for _ in range(max(0, min(3, tgt - emitted))):
                        do_epilogue(emitted)
                        emitted += 1
            while emitted < NBLK:
                do_epilogue(emitted)
                emitted += 1

        # ---- head ----------------------------------------------------------
        for off, cw in CHUNKS:
            cw = min(cw, NC_N - off)
            ps = ps_dense.tile([128, 512], FP, tag="dense")
            nc.tensor.matmul(
                ps[:OUT_D, :cw], w_sb["Wh"][:], h_sb[:, off : off + cw]
            )
            ot = oc_p.tile([OUT_D, 512], FP, tag="ot")
            nc.scalar.activation(
                ot[:, :cw], ps[:OUT_D, :cw], ACT_F.Identity,
                bias=bias[:OUT_D, 14:15],
            )
            nc.sync.dma_start(out_d[:, off : off + cw], ot[:, :cw])

    nc.compile()
    return nc


def _make_in_maps(inputs, per_core):
    import ml_dtypes

    x = np.asarray(inputs["x"], dtype=np.float32)
    # the W1/W2 MLP layers store h+1 (ELU plus one); the constant offset is
    # folded into the consuming layer's bias via column sums of the bf16
    # weights actually used on device
    w2bf = np.asarray(inputs["W2"], np.float32).astype(ml_dtypes.bfloat16)
    w3bf = np.asarray(inputs["W3"], np.float32).astype(ml_dtypes.bfloat16)
    bias = np.zeros((128, 24), dtype=np.float32)
    for j, nm in enumerate(["b1", "b2", "b3", "bg1", "bg2", "bg3", "bg4"]):
        b = np.asarray(inputs[nm], dtype=np.float32)
        bias[:, j] = b
        bias[:, j + 16] = b - 1.0
    bias[:, 1] -= w2bf.astype(np.float32).sum(axis=0)
    bias[:, 2] -= w3bf.astype(np.float32).sum(axis=0)
    bias[:, 17] = bias[:, 1] - 1.0
    bias[:, 18] = bias[:, 2] - 1.0
    bias[:OUT_D, 14] = np.asarray(inputs["bh"], dtype=np.float32)

    shared = {
        "bias": bias,
        "iota128": np.tile(
            np.arange(BW, dtype=np.float32), (128, 1)
        ).astype(ml_dtypes.bfloat16),
    }
    for nm in ["W1", "W2", "W3", "Wg1", "Wg2", "Wg3", "Wg4", "Wh"]:
        shared[nm] = np.ascontiguousarray(
            np.asarray(inputs[nm], np.float32)
        ).astype(ml_dtypes.bfloat16)

    in_maps = []
    for c in range(P):
        m = dict(shared)
        m["xT"] = np.ascontiguousarray(
            x[c * NC_N : (c + 1) * NC_N].T
        ).astype(ml_dtypes.bfloat16)
        m.update(per_core[c])
        in_maps.append(m)
    return in_maps


def run(inputs, trace=False):
    """Run the distributed kernel; returns (out [N, OUT_D] fp32, results)."""
    tcnt, per_core = _prep_edges(inputs["edge_index"], inputs["edge_weight"])
    nc = _build_program(tcnt)
    in_maps = _make_in_maps(inputs, per_core)
    res = run_bass_kernel_spmd(nc, in_maps, list(range(P)), trace=trace)
    out = np.concatenate(
        [res.results[c]["out"].T for c in range(P)], axis=0
    ).astype(np.float32)
    return out, res


def kernel(**inputs):
    out, _ = run(inputs, trace=False)
    return out

